# revision 13
# baseline (speedup 1.0000x reference)
"""Multi-head attention block (nn_Attention) on 8 Trainium2 NeuronCores.

Reference computation (per batch element, all fp32):
    qkv = x @ w_qkv.T + b_qkv               # [T=1024, 3D], D=768
    q, k, v per head (H=12, Hd=64)
    attn = softmax(q @ k.T / sqrt(Hd))
    out  = (attn @ v) @ w_proj.T + b_proj   # [T, D]

Sharding: pure data parallelism over the batch (B=8) — one batch element per
NeuronCore, weights replicated, no collectives.

The wall-setting engine is ScalarE: softmax needs 12M exp evaluations per
core at 1 elem/lane/cycle @ 1.2 GHz (~82 us) plus a ~200-cycle SBUF-access
overhead per activation instruction. This version reduces the instruction
count from 96 to 64 by restructuring PSUM so each exp call covers a
[128, 1536] tile (3 PSUM banks):
  - score staging s_ps = 2 bufs x 3 banks (6 banks). Each buf holds 3
    "units" [128 keys, 512 queries] of the (head-pair, chunk, key-tile,
    head-parity) stream; one exp instruction converts a full buf to
    probabilities (bf16, into SBUF).
  - the attn@v accumulators for the two heads of a pair share ONE bank by
    running as two sequential passes (head 2i fully accumulated +
    normalized, then head 2i+1 re-using the same bank). Probabilities are
    buffered in SBUF (bf16) until the second pass.
  - qkv/proj matmul evacuation staging mm_ps = 1 bank.
Probabilities, V, the attention output OT, and w_proj are held in bf16
(~0.4% quantization each, well inside the 2e-2 tolerance); matmuls run at
the same PE rate as f32r.

Per-core pipeline (phases overlap; the Tile scheduler is priority-ordered by
emission): qkT [1536, T] features stream as fillers under the exp stream;
V staged token-major as [v_h | 1] blocks of 65 columns (the ones column
makes the attention matmul emit the softmax denominators for free as row
64); S.T = kT_h.T @ qT_h per 128-key tile (head pairs at partition bases
0/64 occupy distinct PE row groups and run concurrently); normalization is
reciprocal (DVE) -> partition_broadcast (GPSIMD) -> multiply (DVE) straight
out of PSUM into OT; projection out = OT.T-contract @ wT_proj + b_proj.
"""
import os

import numpy as np

os.environ.setdefault("JAX_COMPILATION_CACHE_DIR", "/tmp/jax_neff_cache")

import concourse.bass as bass
import concourse.bacc as bacc
import concourse.tile as tile
from concourse import mybir

F32 = mybir.dt.float32
F32R = mybir.dt.float32r
BF16 = mybir.dt.bfloat16

B, T, D = 8, 1024, 768
H, HD = 12, 64
SCALE = HD ** -0.5
N_CORES = 8
TT = T // 128       # 8 key tiles
DT = D // 128       # 6 contraction tiles
TQ = 512            # query chunk (moving dim)
NCH = T // TQ       # 2 query chunks
GSZ = 3             # units per exp group (3 x 512 q = one 3-bank PSUM buf)


def _bcast_ap(ap_1d, parts, n):
    return bass.AP(tensor=ap_1d.tensor, offset=ap_1d.offset,
                   ap=[[0, parts], [1, n]])


def build_nc(reps=1, debug_taps=False):
    nc = bacc.Bacc(trn_type="TRN2", debug=False, num_devices=N_CORES)
    xt_d = nc.dram_tensor("xT", (D, T), F32, kind="ExternalInput")
    wqkv_d = nc.dram_tensor("wT_qkv", (D, 3 * D), F32, kind="ExternalInput")
    bqkv_d = nc.dram_tensor("b_qkv", (3 * D,), F32, kind="ExternalInput")
    wproj_d = nc.dram_tensor("wT_proj", (D, D), BF16, kind="ExternalInput")
    bproj_d = nc.dram_tensor("b_proj", (D,), F32, kind="ExternalInput")
    out_d = nc.dram_tensor("out", (T, D), F32, kind="ExternalOutput")
    taps = None
    if debug_taps:
        taps = {
            "tap_qk0": nc.dram_tensor("tap_qk0", (128, T), F32, kind="ExternalOutput"),
            "tap_qk6": nc.dram_tensor("tap_qk6", (128, T), F32, kind="ExternalOutput"),
            "tap_pt0": nc.dram_tensor("tap_pt0", (128, GSZ, TQ), BF16,
                                      kind="ExternalOutput"),
            "tap_ot0": nc.dram_tensor("tap_ot0", (128, T), BF16,
                                      kind="ExternalOutput"),
            "tap_v0": nc.dram_tensor("tap_v0", (128, H, 66), BF16,
                                     kind="ExternalOutput"),
        }

    with tile.TileContext(nc) as tc:
        for _ in range(reps):
            _body(nc, tc, xt_d, wqkv_d, bqkv_d, wproj_d, bproj_d, out_d, taps)
    nc.compile()
    return nc


def _body(nc, tc, xt_d, wqkv_d, bqkv_d, wproj_d, bproj_d, out_d, taps=None):
    from contextlib import ExitStack
    with ExitStack() as ctx:
        consts = ctx.enter_context(tc.tile_pool(name="consts", bufs=1))
        qkt_pool = ctx.enter_context(tc.tile_pool(name="qkt", bufs=1))
        v_pool = ctx.enter_context(tc.tile_pool(name="vst", bufs=1))
        ot_pool = ctx.enter_context(tc.tile_pool(name="ot", bufs=1))
        x_pool = ctx.enter_context(tc.tile_pool(name="x", bufs=1))
        wqk_pool = ctx.enter_context(tc.tile_pool(name="wqk", bufs=1))
        mm_ps = ctx.enter_context(tc.tile_pool(name="mmps", bufs=1, space="PSUM"))

        bias_qk = consts.tile([128, 12], F32)
        nc.sync.dma_start(bias_qk[:], bqkv_d[0:1536].rearrange("(t p) -> p t", p=128))
        bias_v = consts.tile([128, D], F32)
        bias_p = consts.tile([128, D], F32)
        ones12 = consts.tile([128, H, 1], BF16)
        nc.vector.memset(ones12[:], 1.0)

        # 66 (not 65) columns per head so each head's slice starts 4-byte
        # aligned in bf16 (65 cols = odd 130-byte stride)
        qkT = [qkt_pool.tile([128, T], F32R, name=f"qkT{fi}") for fi in range(12)]
        vst = [v_pool.tile([128, H, 66], BF16, name=f"vst{ti}") for ti in range(TT)]
        OT = [ot_pool.tile([128, T], BF16, name=f"OT{k}") for k in range(DT)]

        xt_r = xt_d.rearrange("(dt p) t -> p dt t", p=128)
        wq_r = wqkv_d.rearrange("(dt p) f -> p dt f", p=128)
        wp_r = wproj_d.rearrange("(dt p) f -> p dt f", p=128)
        xTM = x_pool.tile([128, DT, T], F32R, name="xTM")
        xT = [xTM[:, k, :] for k in range(DT)]
        wTqkM = wqk_pool.tile([128, DT, 1536], F32R, name="wTqkM")
        wTqk = [wTqkM[:, k, :] for k in range(DT)]
        for k in range(DT):
            nc.scalar.dma_start(
                xTM[:, k, 0:512], xt_r[:, k, 0:512].bitcast(F32R))
        for k in range(DT):
            nc.scalar.dma_start(
                xTM[:, k, 512:1024], xt_r[:, k, 512:1024].bitcast(F32R))

        def emit_fi_chunk(fi, c, with_dma):
            """One qkT feature tile, one 512-token chunk; optionally stream
            the wT_qkv column slice first."""
            if with_dma:
                for k in range(DT):
                    nc.sync.dma_start(
                        wTqkM[:, k, 128 * fi:128 * (fi + 1)],
                        wq_r[:, k, 128 * fi:128 * (fi + 1)].bitcast(F32R))
            pq = mm_ps.tile([128, TQ], F32, tag="mm", name=f"pq{fi}_{c}")
            for k in range(DT):
                nc.tensor.matmul(
                    pq[:], wTqk[k][:, 128 * fi:128 * (fi + 1)],
                    xT[k][:, TQ * c:TQ * (c + 1)],
                    start=(k == 0), stop=(k == DT - 1))
            nc.vector.tensor_scalar_add(
                qkT[fi][:, TQ * c:TQ * (c + 1)], pq[:], bias_qk[:, fi:fi + 1])

        def emit_fi(fi):
            for c in range(NCH):
                emit_fi_chunk(fi, c, with_dma=(c == 0))

        def emit_v_half(c2, wTv, ti_range=None):
            """v columns [384*c2, 384*(c2+1)) for token tiles (heads 6c2..6c2+6)."""
            for ti in (ti_range if ti_range is not None else range(TT)):
                pv = mm_ps.tile([128, 384], F32, tag="mm", name=f"pv{ti}_{c2}")
                for k in range(DT):
                    nc.tensor.matmul(
                        pv[:], xT[k][:, 128 * ti:128 * (ti + 1)],
                        wTv[k][:], start=(k == 0), stop=(k == DT - 1))
                nc.vector.tensor_add(
                    vst[ti][:, 6 * c2:6 * (c2 + 1), 0:64],
                    pv[:].rearrange("p (h d) -> p h d", d=64),
                    bias_v[:, 384 * c2:384 * (c2 + 1)].rearrange(
                        "p (h d) -> p h d", d=64))
                nc.vector.tensor_copy(
                    vst[ti][:, 6 * c2:6 * (c2 + 1), 64:65], ones12[:, 0:6, :])

        wshare = ctx.enter_context(tc.tile_pool(name="wshare", bufs=2))

        def load_wv(c2, eng=None):
            eng = eng or nc.sync
            m = wshare.tile([128, DT, 384], F32R, tag="ws", name=f"wTvM{c2}")
            for k in range(DT):
                eng.dma_start(
                    m[:, k, :],
                    wq_r[:, k, 1536 + 384 * c2:1536 + 384 * (c2 + 1)].bitcast(F32R))
            return [m[:, k, :] for k in range(DT)]

        # ---------------- attention machinery ----------------
        attn_ctx = ExitStack()
        s_ps = attn_ctx.enter_context(tc.tile_pool(name="sps", bufs=2, space="PSUM"))
        pt_pool = attn_ctx.enter_context(tc.tile_pool(name="pt", bufs=8))
        o_ps = attn_ctx.enter_context(tc.tile_pool(name="ops", bufs=1, space="PSUM"))
        sst_pool = attn_ctx.enter_context(tc.tile_pool(name="sst", bufs=2))
        rsb_pool = attn_ctx.enter_context(tc.tile_pool(name="rsb", bufs=2))

        # global unit stream: (hp, c, tkt, p); groups of GSZ share one
        # s_ps buf + one exp instruction.
        units = [(hp, c, tkt, p)
                 for hp in range(6) for c in range(NCH)
                 for tkt in range(TT) for p in (0, 1)]
        groups = [units[i:i + GSZ] for i in range(0, len(units), GSZ)]

        # per-group psum/sbuf tiles, filled by emit_group_s / flushed by exp
        sp_tiles = {}
        pt_tiles = {}

        def emit_group_s(g):
            sp = s_ps.tile([128, GSZ, TQ], F32, tag="s", name=f"sp{g}")
            sp_tiles[g] = sp
            for slot, (hp, c, tkt, p) in enumerate(groups[g]):
                nc.tensor.matmul(
                    sp[:, slot, :],
                    qkT[6 + hp][64 * p:64 * (p + 1), 128 * tkt:128 * (tkt + 1)],
                    qkT[hp][64 * p:64 * (p + 1), TQ * c:TQ * (c + 1)],
                    start=True, stop=True)

        def emit_group_exp(g):
            pt = pt_pool.tile([128, GSZ, TQ], BF16, tag="pt", name=f"pt{g}")
            pt_tiles[g] = pt
            nc.scalar.activation(
                pt[:, :, :], sp_tiles[g][:, :, :],
                mybir.ActivationFunctionType.Exp, bias=0.0, scale=float(SCALE))
            if taps is not None and g == 0:
                nc.sync.dma_start(taps["tap_pt0"][:], pt[:, :, :])

        # delivery state per (hp, c)
        po_cur = {}         # (hp,c) -> pass-A psum tile
        pend_p1 = {}        # (hp,c) -> list of (tkt, pt_slice)
        on_c_done = {}      # (hp,c) -> callables (fillers) after that iter

        def emit_norm(hp, c, p, po):
            # reciprocal_approx_fast is a custom-DVE op that re-reads its
            # input; PSUM has a single DVE read port, so bounce the
            # denominator row through SBUF first.
            sst = sst_pool.tile([1, TQ], F32, tag="sst", name=f"sst{hp}_{c}_{p}")
            nc.vector.tensor_copy(sst[0:1, :], po[64:65, :])
            nc.vector.reciprocal_approx_fast(sst[0:1, :], sst[0:1, :])
            rsb = rsb_pool.tile([64, TQ], F32, tag="rsb", name=f"rsb{hp}_{c}_{p}")
            nc.gpsimd.partition_broadcast(rsb[:], sst[0:1, :])
            nc.vector.tensor_mul(
                OT[hp][64 * p:64 * (p + 1), TQ * c:TQ * (c + 1)],
                po[0:64, :], rsb[:])

        def deliver_group(g):
            for slot, (hp, c, tkt, p) in enumerate(groups[g]):
                pts = pt_tiles[g][:, slot, :]
                if p == 0:
                    if (hp, c) not in po_cur:
                        po_cur[(hp, c)] = o_ps.tile(
                            [128, TQ], F32, tag="o", name=f"poA{hp}_{c}")
                        pend_p1[(hp, c)] = []
                    nc.tensor.matmul(
                        po_cur[(hp, c)][0:65, :], vst[tkt][:, 2 * hp, 0:65], pts,
                        start=(tkt == 0), stop=(tkt == TT - 1),
                        skip_group_check=True)
                else:
                    pend_p1[(hp, c)].append((tkt, pts))
                if tkt == TT - 1 and p == 1:
                    # pass A complete (all p0 units precede (7,1) in stream)
                    emit_norm(hp, c, 0, po_cur[(hp, c)])
                    poB = o_ps.tile([128, TQ], F32, tag="o", name=f"poB{hp}_{c}")
                    for btk, bpts in pend_p1[(hp, c)]:
                        nc.tensor.matmul(
                            poB[0:65, :], vst[btk][:, 2 * hp + 1, 0:65], bpts,
                            start=(btk == 0), stop=(btk == TT - 1),
                            skip_group_check=True)
                    emit_norm(hp, c, 1, poB)
                    del po_cur[(hp, c)], pend_p1[(hp, c)]
                    for job in on_c_done.pop((hp, c), []):
                        job()

        # deferred fillers, fired when each head pair's attention completes
        wTv1_box = {}
        wTp = {}

        def load_wv1():
            wTv1_box["w"] = load_wv(1)

        def load_wp():
            for c2 in range(2):
                m = wshare.tile([128, DT, 384], BF16, tag="ws", name=f"wTpM{c2}")
                for k in range(DT):
                    nc.sync.dma_start(
                        m[:, k, :], wp_r[:, k, 384 * c2:384 * (c2 + 1)])
                    wTp[(c2, k)] = m[:, k, :]

        # fillers fired as the attention stream completes each (hp, c):
        # the NEXT head pair's q/k features must be emitted before its
        # S-matmuls are (emission order defines write->read binding).
        on_c_done[(0, 0)] = [lambda: emit_fi(1), lambda: emit_fi(7)]
        on_c_done[(0, 1)] = [load_wv1]
        on_c_done[(1, 0)] = [lambda: emit_fi(2), lambda: emit_fi(8)]
        on_c_done[(1, 1)] = [lambda: emit_v_half(1, wTv1_box["w"],
                                                 ti_range=range(0, 4))]
        on_c_done[(2, 0)] = [lambda: emit_fi(3), lambda: emit_fi(9)]
        on_c_done[(2, 1)] = [lambda: emit_v_half(1, wTv1_box["w"],
                                                 ti_range=range(4, TT))]
        on_c_done[(3, 0)] = [lambda: emit_fi(4), lambda: emit_fi(10)]
        on_c_done[(4, 0)] = [lambda: emit_fi(5), lambda: emit_fi(11)]
        on_c_done[(4, 1)] = [load_wp]

        # ---------------- lead-in fillers + pipeline ----------------
        emit_fi_chunk(0, 0, with_dma=True)
        emit_fi_chunk(6, 0, with_dma=True)
        wTv0_box = {}

        def lead1():
            emit_fi_chunk(0, 1, with_dma=False)
            emit_fi_chunk(6, 1, with_dma=False)

        def lead2():
            nc.sync.dma_start(bias_v[:], _bcast_ap(bqkv_d[1536:2304], 128, D))
            nc.sync.dma_start(bias_p[:], _bcast_ap(bproj_d[0:D], 128, D))
            wTv0_box["w"] = load_wv(0, eng=nc.gpsimd)

        lead_jobs = {
            1: [lead1],
            2: [lead2, lambda: emit_v_half(0, wTv0_box["w"], ti_range=range(0, 2))],
            3: [lambda: emit_v_half(0, wTv0_box["w"], ti_range=range(2, 4))],
            4: [lambda: emit_v_half(0, wTv0_box["w"], ti_range=range(4, 6))],
            5: [lambda: emit_v_half(0, wTv0_box["w"], ti_range=range(6, 8))],
        }

        ng = len(groups)
        for g in range(ng + 2):
            if g < ng:
                emit_group_s(g)
            for job in lead_jobs.pop(g, []):
                job()
            if 1 <= g <= ng:
                emit_group_exp(g - 1)
            if g >= 2:
                deliver_group(g - 2)
        attn_ctx.close()

        # ---------------- projection ----------------
        with ExitStack() as ctx3:
            outst = ctx3.enter_context(tc.tile_pool(name="outst", bufs=3))
            for ti in range(TT):
                ob = outst.tile([128, D], F32, tag="ob", name=f"ob{ti}")
                for c2 in range(2):
                    pp = mm_ps.tile([128, 384], F32, tag="mm", name=f"pp{ti}_{c2}")
                    for k in range(DT):
                        nc.tensor.matmul(
                            pp[:], OT[k][:, 128 * ti:128 * (ti + 1)],
                            wTp[(c2, k)][:],
                            start=(k == 0), stop=(k == DT - 1))
                    nc.vector.tensor_add(
                        ob[:, 384 * c2:384 * (c2 + 1)], pp[:],
                        bias_p[:, 384 * c2:384 * (c2 + 1)])
                nc.sync.dma_start(out_d[128 * ti:128 * (ti + 1), :], ob[:])
            if taps is not None:
                nc.sync.dma_start(taps["tap_qk0"][:], qkT[0][:].bitcast(F32))
                nc.sync.dma_start(taps["tap_qk6"][:], qkT[6][:].bitcast(F32))
                nc.sync.dma_start(taps["tap_ot0"][:], OT[0][:])
                nc.sync.dma_start(taps["tap_v0"][:], vst[0][:])


_CACHE = {}


def _get_runner():
    if "runner" in _CACHE:
        return _CACHE["runner"]
    import jax
    from jax.sharding import Mesh, PartitionSpec
    from jax.experimental.shard_map import shard_map
    from concourse import bass2jax
    from concourse.bass2jax import _bass_exec_p, partition_id_tensor

    nc = build_nc()
    bass2jax.install_neuronx_cc_hook()
    partition_name = nc.partition_id_tensor.name if nc.partition_id_tensor else None
    in_names, out_names, out_avals = [], [], []
    for alloc in nc.m.functions[0].allocations:
        if not isinstance(alloc, mybir.MemoryLocationSet):
            continue
        name = alloc.memorylocations[0].name
        if alloc.kind == "ExternalInput":
            if name != partition_name:
                in_names.append(name)
        elif alloc.kind == "ExternalOutput":
            out_names.append(name)
            out_avals.append(jax.core.ShapedArray(
                tuple(alloc.tensor_shape), mybir.dt.np(alloc.dtype)))
    all_in = list(in_names) + list(out_names)
    if partition_name is not None:
        all_in.append(partition_name)

    def _jbody(*args):
        operands = list(args)
        if partition_name is not None:
            operands.append(partition_id_tensor())
        return tuple(_bass_exec_p.bind(
            *operands, out_avals=tuple(out_avals), in_names=tuple(all_in),
            out_names=tuple(out_names), lowering_input_output_aliases=(),
            sim_require_finite=True, sim_require_nnan=True, nc=nc))

    devices = jax.devices()[:N_CORES]
    mesh = Mesh(np.asarray(devices), ("core",))
    # xT is batch-sharded on the core axis; weights/biases are replicated.
    sharded_in = {"xT"}
    in_specs = tuple(
        PartitionSpec("core") if n in sharded_in else PartitionSpec()
        for n in in_names
    ) + (PartitionSpec("core"),) * len(out_names)
    fn = jax.jit(
        shard_map(_jbody, mesh=mesh, in_specs=in_specs,
                  out_specs=(PartitionSpec("core"),) * len(out_names),
                  check_rep=False),
        keep_unused=True)
    _CACHE["runner"] = (fn, in_names, out_names, out_avals, mesh)
    return _CACHE["runner"]


def _weight_key(*arrs):
    import hashlib
    h = hashlib.sha1()
    for a in arrs:
        h.update(np.ascontiguousarray(a, np.float32).tobytes())
    return h.hexdigest()


def host_inputs(x0T, w_qkv, b_qkv, w_proj, b_proj):
    """Per-core DRAM tensor dict for one batch element (timing helper)."""
    bf16 = mybir.dt.np(BF16)
    return {
        "xT": np.ascontiguousarray(x0T, np.float32),
        "wT_qkv": np.ascontiguousarray(np.asarray(w_qkv, np.float32).T),
        "b_qkv": np.asarray(b_qkv, np.float32),
        "wT_proj": np.ascontiguousarray(
            np.asarray(w_proj, np.float32).T.astype(bf16)),
        "b_proj": np.asarray(b_proj, np.float32),
    }


def kernel(x, w_qkv, b_qkv, w_proj, b_proj):
    import jax
    fn, in_names, out_names, out_avals, mesh = _get_runner()
    x = np.asarray(x, dtype=np.float32)
    xt = np.ascontiguousarray(np.transpose(x, (0, 2, 1)))        # [B, D, T]
    xt_flat = xt.reshape(N_CORES * D, T)

    wk = _weight_key(w_qkv, b_qkv, w_proj, b_proj)
    if _CACHE.get("wkey") != wk:
        bf16 = mybir.dt.np(BF16)
        wqt = np.ascontiguousarray(np.asarray(w_qkv, np.float32).T)   # [D, 3D]
        wpt = np.ascontiguousarray(
            np.asarray(w_proj, np.float32).T.astype(bf16))            # [D, D] bf16
        host_w = {
            "wT_qkv": wqt,
            "b_qkv": np.asarray(b_qkv, np.float32),
            "wT_proj": wpt,
            "b_proj": np.asarray(b_proj, np.float32),
        }
        _CACHE["wdev"] = {k: jax.device_put(v) for k, v in host_w.items()}
        _CACHE["wkey"] = wk
    wdev = _CACHE["wdev"]

    args = []
    for n in in_names:
        args.append(xt_flat if n == "xT" else wdev[n])
    for a in out_avals:
        args.append(np.zeros((N_CORES * a.shape[0], *a.shape[1:]), a.dtype))
    outs = fn(*args)
    jax.block_until_ready(outs)
    oi = out_names.index("out")
    return np.asarray(outs[oi]).reshape(N_CORES, T, D).astype(np.float32)
